# revision 42
# baseline (speedup 1.0000x reference)
"""Multi-head attention (RoPE, causal) on 8 TRN2 NeuronCores.

Sharding: DP2 x TP4. Core c handles batch b = c//4 and heads
H_c = {4*(c%4) .. 4*(c%4)+3}. Attention outputs are exchanged with two
per-pair 8-rank AllToAlls (bf16), after which every core computes the
final out-projection for a 256-row q-slice of BOTH batches with the full
head dimension locally. The host-side unshard is a pure concatenation.

Structure (measures ~207-225us incl rank-start skew; PE envelope ~166us):
  - All device tensors host-packed into partition-major contiguous
    layouts (x pre-transposed on host) so every load is one cheap DMA.
    DMA issue order tracks first use: pair-0 Q/K weights and the first
    half of x chunk 0 land first, so the first matmul starts ~13us in
    (limited by the ~11us engine preamble).
  - Projection (PE-dense, full-array) and attention (ScalarE-dense
    softmax) are software-pipelined one chunk deep and interleaved at
    unit granularity: projection chunk sc weaves between the attention
    units of q-chunk sc-1, so exp always has scores to chew on, the PE
    fills exp-bound gaps with QKV chains, and the array activity stays
    high enough to keep the HAM clock-gate warm.
  - scores for the two heads of a pair are emitted adjacently into the
    two banks of one [128,1024] PSUM tile; their K=64 matmuls carry
    tile_position (0,0)/(64,0) (complementary PE row groups), run
    CONCURRENTLY (measured 196ns/matmul issue rate), and ONE Exp
    activation covers both heads.
  - causal fine-triangle mask multiplies p post-exp on GpSimd — except
    pair-1 q-chunk-3 (between the exchanges) which masks on DVE. A
    pre-exp additive mask on PSUM measured worse: it put DVE in the
    scores->exp critical chain (and GpSimd cannot touch PSUM).
  - softmax denominator: ones-column folded into V (the ones column is
    memset once; the V projection writes only the 64 v-columns, and V
    bias is zero per the spec so there is no bias matmul); d broadcast
    by a K=1 matmul, then reciprocal_approx_fast. (GpSimd
    partition_broadcast measured ~7us/call on HW — do not use it.)
  - attention drains per-head: head 0's epilogue + AllToAll pack are
    not serialized behind head 1's last av matmuls.
  - pair 0 finishes first so its AllToAll overlaps pair 1's last
    chunk; the gathered-attn loads are issued inside exchange() so
    they fire the moment the collective lands. Phase C is p-major
    (pair-0 accumulation overlaps the second AllToAll) with per-slot
    fused drain+store, bf16 output staging. All 8 bias matmuls and a
    few dependency-free warm matmuls are hoisted ahead of the
    af-dependent accumulations, so the in-order PE queue does not
    head-of-line block during the first-AllToAll wait and the p-state
    stays at full clock.
  - fp8 (DoubleRow) for V/av/p was tried and REVERTED: softmax
    averaging shrinks the attention output ~20x below the v scale, but
    not the quantization noise — p-fp8 and v-fp8 each cost ~2.7%
    relative output error (measured 5% combined vs the 2% budget).
"""

import sys

for _p in ("/opt/trn_rl_repo",):
    if _p not in sys.path:
        sys.path.insert(0, _p)

import numpy as np
import ml_dtypes

from concourse import bacc, bass, mybir, tile
from concourse.bass_utils import run_bass_kernel_spmd

F32 = mybir.dt.float32
BF16 = mybir.dt.bfloat16
FP8 = mybir.dt.float8e4

D, H, HD, S, B = 1024, 16, 64, 2048, 2
HPC = 4          # heads per core
NP = 2           # head pairs per core
QC = 512         # q-chunk size
KB = 128         # k-block size
NQC = S // QC    # 4
NKB = S // KB    # 16
NC = 8           # total cores; the AllToAll spans all 8
SLC = S // NC    # 256 rows of final output per core (for BOTH batches)
QPR = QC // NC   # 64: q-cols per rank in one chunked AllToAll

Ident = mybir.ActivationFunctionType.Identity
Exp = mybir.ActivationFunctionType.Exp

# --- packed-constants layout (columns of the [128, CW] bf16 tensor) ----
# pair-0 Q/K weights first (needed by the very first projection unit),
# then rope/misc consts, then pair-1 and V weights.
OFF_WQ0 = 0                  # [128, 8, 128] pair-0 Q
OFF_WK0 = OFF_WQ0 + 8 * 128  # [128, 8, 128] pair-0 K
OFF_MISC = OFF_WK0 + 8 * 128
OFF_PERM = OFF_MISC          # [128, 128]
OFF_ONES = OFF_PERM + 128    # [128, 128] all-ones
OFF_TRIN = OFF_ONES + 128    # [128, 128] additive causal mask (0 / -1e30)
OFF_COS = OFF_TRIN + 128     # [128, 2048]
OFF_SIN = OFF_COS + S        # [128, 2048]
OFF_ROW = OFF_SIN + S        # row-0 data: ones[512] | bv[260] | bo[1024]
OFF_ROW_ONES = OFF_ROW
OFF_ROW_BV = OFF_ROW_ONES + 512
OFF_ROW_BO = OFF_ROW_BV + 65 * HPC
OFF_WQ1 = OFF_ROW_BO + D     # [128, 8, 128] pair-1 Q
OFF_WK1 = OFF_WQ1 + 8 * 128  # [128, 8, 128] pair-1 K
OFF_WV = OFF_WK1 + 8 * 128   # [128, 8, 256] V (no ones-padding columns)
CW = OFF_WV + 8 * 256

# exp shift: p = exp(scale*s - EXP_SHIFT). Keeps fp8e4 p below max-normal
# 240 (softmax is shift-invariant; numerator and denominator both scale).
# Measured max scaled causal score is 5.85 -> p_max ~ e^4.85 = 128.
EXP_SHIFT = 1.0
DR = mybir.MatmulPerfMode.DoubleRow


def _host_constants():
    pos = np.arange(S, dtype=np.float64)
    inv_freq = 1.0 / (10000.0 ** (np.arange(0, HD, 2, dtype=np.float64) / HD))
    freqs = np.outer(pos, inv_freq)
    cosT = np.repeat(np.cos(freqs), 2, axis=1).T.astype(np.float32)  # [64, S]
    sinT = np.repeat(np.sin(freqs), 2, axis=1).T.astype(np.float32)
    # pair-stacked: same table on both 64-partition halves
    cosT = np.concatenate([cosT, cosT], axis=0)  # [128, S]
    sinT = np.concatenate([sinT, sinT], axis=0)
    perm = np.zeros((128, 128), dtype=np.float32)
    for base in (0, 64):
        for i in range(32):
            perm[base + 2 * i + 1, base + 2 * i] = -1.0
            perm[base + 2 * i, base + 2 * i + 1] = 1.0
    # causal fine triangle for a 128-col diagonal slice: keep q >= k
    tri = (np.arange(128)[None, :] >= np.arange(128)[:, None]).astype(np.float32)
    return cosT, sinT, perm, tri


def build_program():
    nc = bacc.Bacc(None, target_bir_lowering=False)

    # --- I/O (all host-packed, partition-major, contiguous) ------------
    # xt: x^T for this core's batch, laid out [128, sc, c, s] so each
    # 512-row s-chunk is one contiguous DMA.
    xt = nc.declare_dram_parameter("xt", [128, NQC, 8, QC], BF16, isOutput=False)
    cw = nc.declare_dram_parameter("cw", [128, CW], BF16, isOutput=False)
    bqk = nc.declare_dram_parameter("bqk", [128, 2 * NP + 1], F32, isOutput=False)
    wo = nc.declare_dram_parameter("wo", [128, 8 * D], BF16, isOutput=False)
    out = nc.declare_dram_parameter("out_s", [B, SLC, D], BF16, isOutput=True)

    with tile.TileContext(nc) as tc:
        with (
            tc.tile_pool(name="persist", bufs=1) as pp,
            tc.tile_pool(name="dram", bufs=1, space="DRAM") as dp,
        ):
            xt_s = pp.tile([128, NQC, 8, QC], BF16)
            cw_s = pp.tile([128, CW], BF16)
            bqk_s = pp.tile([128, 2 * NP + 1], F32)
            wo_s = pp.tile([128, 8 * D], BF16)

            # issue order tracks first-use: pair-0 QK weights, first s-chunk
            # (split in two so the Q chain can start on the first half),
            # rope/misc consts, V weights+inputs (fp8), pair-1 weights, rest
            # of x, out-proj weights.
            nc.sync.dma_start(
                out=cw_s[:, OFF_WQ0:OFF_WK0], in_=cw[:, OFF_WQ0:OFF_WK0]
            )
            nc.sync.dma_start(out=xt_s[:, 0, 0:4], in_=xt[:, 0, 0:4])
            nc.sync.dma_start(
                out=cw_s[:, OFF_WK0:OFF_MISC], in_=cw[:, OFF_WK0:OFF_MISC]
            )
            nc.sync.dma_start(out=xt_s[:, 0, 4:8], in_=xt[:, 0, 4:8])
            nc.sync.dma_start(out=bqk_s[:], in_=bqk[:])
            nc.sync.dma_start(
                out=cw_s[:, OFF_MISC:OFF_WQ1], in_=cw[:, OFF_MISC:OFF_WQ1]
            )
            # V weights land before pair-1 Q/K: the V units (3rd-6th of
            # each projection chunk) need them ~1.5us before the pair-1
            # chains do.
            nc.sync.dma_start(out=cw_s[:, OFF_WV:CW], in_=cw[:, OFF_WV:CW])
            nc.sync.dma_start(
                out=cw_s[:, OFF_WQ1:OFF_WV], in_=cw[:, OFF_WQ1:OFF_WV]
            )
            for sc in range(1, NQC):
                nc.sync.dma_start(out=xt_s[:, sc], in_=xt[:, sc])
            nc.sync.dma_start(out=wo_s[:], in_=wo[:])

            wq_p = [
                cw_s[:, o : o + 8 * 128].rearrange("p (c n) -> p c n", c=8)
                for o in (OFF_WQ0, OFF_WQ1)
            ]
            wk_p = [
                cw_s[:, o : o + 8 * 128].rearrange("p (c n) -> p c n", c=8)
                for o in (OFF_WK0, OFF_WK1)
            ]
            wv_s = cw_s[:, OFF_WV : OFF_WV + 8 * 256].rearrange(
                "p (c n) -> p c n", c=8
            )
            perm_s = cw_s[:, OFF_PERM : OFF_PERM + 128]
            ones_f = cw_s[:, OFF_ONES : OFF_ONES + 128]
            trin_s = cw_s[:, OFF_TRIN : OFF_TRIN + 128]
            cos_s = cw_s[:, OFF_COS : OFF_COS + S]
            sin_s = cw_s[:, OFF_SIN : OFF_SIN + S]
            bv_s = cw_s[0:1, OFF_ROW_BV : OFF_ROW_BV + 65 * HPC]
            bo_s = cw_s[0:1, OFF_ROW_BO : OFF_ROW_BO + D]
            wo_v = wo_s[:].rearrange("p (c n) -> p c n", c=8)

            # persistent activations
            qt = pp.tile([128, NP * S], BF16)   # rotated Q^T, pair-major
            kt = pp.tile([128, NP * S], BF16)   # rotated K^T
            # vt[kb]: [128 srows, head slot * 65]; col 64 of each 65-slot is
            # the all-ones denominator column, set once here by memset (the
            # V projection writes only the 64 v columns).
            vt = [pp.tile([128, HPC * 65], BF16, name=f"vt{i}") for i in range(NKB)]
            for t in vt:
                nc.gpsimd.memset(
                    t[:].rearrange("p (s c) -> p s c", s=HPC)[:, :, 64:65], 1.0
                )
            # attnT[p]: [64, 2*S] — within-pair head h at cols [S*h, S*(h+1))
            attnT = [pp.tile([64, NP * S], BF16, name=f"attnT{p}") for p in range(NP)]
            # gathered attn^T for my q-slice: [hd(128), b2, k-chunk, q(256)]
            af_all = pp.tile([128, B, 8, SLC], BF16)

            # DRAM bounce buffers for the per-pair 8-rank AllToAll
            cc_in = [
                dp.tile([NC, 128, SLC], BF16, name=f"cci{i}") for i in range(NP)
            ]
            cc_out = [
                dp.tile([NC, 128, SLC], BF16, name=f"cco{i}") for i in range(NP)
            ]

            # 1KB warm-up AllToAll: the FIRST collective pays a constant
            # ~11.5us ncfw pickup delay (later ones ~1-2us). Absorb it — and
            # some rank-start skew — here, while GpSimd is otherwise idle
            # for the first ~25us (collective_compute blocks its queue).
            cc_wu_in = dp.tile([NC, 1, 64], BF16)
            cc_wu_out = dp.tile([NC, 1, 64], BF16)
            nc.sync.dma_start(
                out=cc_wu_in[:],
                in_=cw_s[0:1, 0:512].rearrange("one (g q) -> g one q", g=NC),
            )
            nc.gpsimd.collective_compute(
                "AllToAll",
                mybir.AluOpType.bypass,
                ins=[cc_wu_in.opt()],
                outs=[cc_wu_out.opt()],
                replica_groups=[[0, 1, 2, 3, 4, 5, 6, 7]],
            )

            # =============================================================
            # Fused projection + attention, one 512-row s-chunk at a time.
            # =============================================================
            with (
                tc.tile_pool(name="qkraw", bufs=4) as rawp,
                tc.tile_pool(name="p_pool", bufs=6) as ppool,
                tc.tile_pool(name="ep_pool", bufs=6) as epool,
                tc.tile_pool(name="pj_psum", bufs=1, space="PSUM") as pjp,
                tc.tile_pool(name="sc_psum", bufs=2, space="PSUM") as scp,
                tc.tile_pool(name="av_psum", bufs=2, space="PSUM") as avp,
            ):
                def project(sc):
                    """Generator: yields after each of its 8 PE chain units.
                    Unit order: Q/K pair 0, V blocks, Q/K pair 1 — so the
                    chain feeding pair-0 attention (and the first AllToAll)
                    completes earliest."""
                    def qk_units(p):
                        for w_s, boff, rot in (
                            (wq_p[p], 0, qt),
                            (wk_p[p], NP, kt),
                        ):
                            ps = pjp.tile([128, 512], F32, tag="pj")
                            for c in range(8):
                                nc.tensor.matmul(
                                    ps[:],
                                    w_s[:, c, :],
                                    xt_s[:, sc, c, :],
                                    start=(c == 0),
                                    stop=(c == 7),
                                )
                            raw = rawp.tile([128, 512], BF16, tag="raw")
                            nc.scalar.activation(
                                raw[:], ps[:], Ident,
                                bias=bqk_s[:, boff + p : boff + p + 1],
                            )
                            pr = pjp.tile([128, 512], F32, tag="rp")
                            nc.tensor.matmul(
                                pr[:], perm_s, raw[:], start=True, stop=True
                            )
                            ssl = slice(QC * sc, QC * sc + QC)
                            dst = rot[:, S * p + QC * sc : S * p + QC * sc + QC]
                            rtmp = rawp.tile([128, 512], BF16, tag="rtmp")
                            nc.vector.tensor_mul(dst, raw[:], cos_s[:, ssl])
                            nc.vector.tensor_mul(rtmp[:], pr[:], sin_s[:, ssl])
                            nc.vector.tensor_add(dst, dst, rtmp[:])
                            yield

                    yield from qk_units(0)
                    # V natural [s, 4*64] for the 4 s-blocks of this chunk
                    # (qkv bias is zero for V here; the ones columns were
                    # memset once at startup, so no bias matmul).
                    for sb in range(4 * sc, 4 * sc + 4):
                        ps = pjp.tile([128, 256], F32, tag="pj")
                        for c in range(8):
                            nc.tensor.matmul(
                                ps[:],
                                xt_s[:, sc, c, 128 * (sb % 4) : 128 * (sb % 4) + 128],
                                wv_s[:, c, :],
                                start=(c == 0),
                                stop=(c == 7),
                            )
                        nc.vector.tensor_copy(
                            vt[sb][:].rearrange("p (s c) -> p s c", s=HPC)[
                                :, :, 0:64
                            ],
                            ps[:].rearrange("p (s c) -> p s c", s=HPC),
                        )
                        yield
                    yield from qk_units(1)

                def attention(p, qc):
                    """Generator: yields after each k-block / epilogue unit."""
                    nkb_q = 4 * qc + 4
                    av = [
                        avp.tile([128, 512], F32, tag="av", name=f"av{p}{qc}{_h}")
                        for _h in range(2)
                    ]
                    def emit_scores(kb):
                        mrel = kb - 4 * qc
                        c0 = 128 * max(mrel, 0)  # first valid q-col
                        sc_ps = scp.tile([128, 1024], F32, tag="sc")
                        for h in range(2):
                            hsl = slice(64 * h, 64 * h + 64)
                            nc.tensor.matmul(
                                sc_ps[:, 512 * h + c0 : 512 * h + 512],
                                kt[hsl, S * p + KB * kb : S * p + KB * kb + KB],
                                qt[
                                    hsl,
                                    S * p + QC * qc + c0 : S * p + QC * qc + 512,
                                ],
                                start=True,
                                stop=True,
                            )
                        p_t = ppool.tile([128, 1024], BF16, tag="p")
                        if c0 == 0:
                            nc.scalar.activation(
                                p_t[:],
                                sc_ps[:],
                                Exp,
                                scale=float(HD**-0.5),
                                bias=bqk_s[:, 2 * NP : 2 * NP + 1],
                            )
                        else:
                            for h in range(2):
                                nc.scalar.activation(
                                    p_t[:, 512 * h + c0 : 512 * h + 512],
                                    sc_ps[:, 512 * h + c0 : 512 * h + 512],
                                    Exp,
                                    scale=float(HD**-0.5),
                                    bias=bqk_s[:, 2 * NP : 2 * NP + 1],
                                )
                        if mrel >= 0:
                            # pair 1 q-chunk 3 runs after exchange(0); keep
                            # its masks off the GpSimd queue (collective).
                            tri_eng = (
                                nc.vector
                                if (p == 1 and qc == NQC - 1)
                                else nc.gpsimd
                            )
                            for h in range(2):
                                tri_eng.tensor_mul(
                                    p_t[:, 512 * h + c0 : 512 * h + c0 + 128],
                                    p_t[:, 512 * h + c0 : 512 * h + c0 + 128],
                                    trin_s,
                                )
                        return (p_t, c0)

                    def emit_av(kb, pts, h):
                        p_t, c0 = pts
                        nc.tensor.matmul(
                            av[h][0:65, c0:512],
                            vt[kb][:, 65 * (2 * p + h) : 65 * (2 * p + h) + 65],
                            p_t[:, 512 * h + c0 : 512 * h + 512],
                            start=(kb == 0),
                            stop=(kb == nkb_q - 1),
                        )

                    def epi(h):
                        # normalize by the softmax denominator (row 64 of av)
                        # and store to attnT as bf16. (A GpSimd
                        # partition_broadcast here measured ~7us per call on
                        # hardware — the K=1 matmul broadcast stays.)
                        # For the LAST q-chunk the chain gates the exchange
                        # trigger, so it runs in 256-col halves: DVE ops are
                        # column-serial, and halving pipelines the
                        # copy->bcast->recip->mul->pack chain (each half is
                        # exactly one dest rank's slice).
                        halves = (
                            (slice(0, 256), slice(256, 512))
                            if qc == NQC - 1
                            else (slice(0, 512),)
                        )
                        bc = pjp.tile([64, 512], F32, tag="rp")
                        rc = epool.tile([64, 512], F32, tag="rc")
                        for hi, sl in enumerate(halves):
                            n = sl.stop - sl.start
                            dr = epool.tile([1, 512], BF16, tag="dr")
                            nc.vector.tensor_copy(dr[0:1, 0:n], av[h][64:65, sl])
                            nc.tensor.matmul(
                                bc[:, sl],
                                ones_f[0:1, 0:64],
                                dr[0:1, 0:n],
                                start=True,
                                stop=True,
                            )
                            nc.vector.reciprocal_approx_fast(rc[:, sl], bc[:, sl])
                            q0 = QC * qc + sl.start
                            nc.vector.tensor_mul(
                                attnT[p][:, S * h + q0 : S * h + q0 + n],
                                av[h][0:64, sl],
                                rc[:, sl],
                            )
                            # pack for the pair's AllToAll now; the exchange
                            # trigger then only waits on the last one.
                            ng = n // 256
                            nc.sync.dma_start(
                                out=cc_in[p].rearrange("g p q -> p g q")[
                                    64 * h : 64 * h + 64,
                                    2 * qc + sl.start // 256
                                    : 2 * qc + sl.start // 256 + ng,
                                ],
                                in_=attnT[p][
                                    :, S * h + q0 : S * h + q0 + n
                                ].rearrange("r (g q) -> r g q", g=ng),
                            )

                    pipe = []
                    for kb in range(nkb_q):
                        pipe.append((kb, emit_scores(kb)))
                        if len(pipe) > 2:
                            kb0, pts0 = pipe.pop(0)
                            emit_av(kb0, pts0, 0)
                            emit_av(kb0, pts0, 1)
                        yield
                    # drain per-head so head 0's epilogue (and its AllToAll
                    # pack) isn't serialized behind head 1's last av matmuls.
                    for h in range(2):
                        for kb0, pts0 in pipe:
                            emit_av(kb0, pts0, h)
                        epi(h)
                        yield

                def exchange(p):
                    # slices were packed per-(qc, h) as their epilogues
                    # completed; just trigger the 8-rank AllToAll.
                    nc.gpsimd.collective_compute(
                        "AllToAll",
                        mybir.AluOpType.bypass,
                        ins=[cc_in[p].opt()],
                        outs=[cc_out[p].opt()],
                        replica_groups=[[0, 1, 2, 3, 4, 5, 6, 7]],
                    )
                    # issue the gathered-attn loads now: they wait on the
                    # collective's semaphore, so phase C's first matmuls can
                    # start the moment the data lands (keeping the PE warm
                    # through the tail instead of stalling behind DMAs issued
                    # after the last attention epilogue).
                    # af_all[:, b2, k, :] with k = 2*g + p sourced from rank
                    # 4*b2 + g of pair p's exchange.
                    for b2 in range(B):
                        nc.sync.dma_start(
                            out=af_all[:, b2, :, :]
                            .rearrange("r (g two) q -> r g two q", two=2)[
                                :, :, p, :
                            ],
                            in_=cc_out[p][4 * b2 : 4 * b2 + 4].rearrange(
                                "g r q -> r g q"
                            ),
                        )

                def drain(gen):
                    for _ in gen:
                        pass

                def interleave(pg, ag, n_proj, n_att):
                    # weave att units between proj units so the ScalarE
                    # (softmax exp) and the PE (projection chains) are both
                    # fed throughout; neither runs dry for long stretches.
                    done_a = 0
                    for i in range(n_proj):
                        if next(pg, StopIteration) is StopIteration:
                            break
                        want = n_att * (i + 1) // n_proj
                        while done_a < want:
                            if next(ag, StopIteration) is StopIteration:
                                done_a = n_att
                                break
                            done_a += 1
                    drain(pg)
                    drain(ag)

                def att_both(qc):
                    yield from attention(0, qc)
                    yield from attention(1, qc)

                def take(gen, n):
                    for _ in range(n):
                        if next(gen, StopIteration) is StopIteration:
                            break
                        yield

                def chain2(g1, g2):
                    yield from g1
                    yield from g2

                # software pipeline, one chunk deep: projection chunk sc
                # interleaves with the attention of q-chunk sc-1 (whose K/V
                # and Q are complete). The last block additionally pulls in
                # pair 0's q-chunk-3 k-blocks 0..11, shrinking the exp-bound
                # tail; pair 0 finishes first so its AllToAll overlaps pair
                # 1's remaining attention.
                # proj(0)'s first 6 units run solo (nothing is ready), but
                # its last two (Q/K pair-1) weave with att(0,0)'s first two
                # k-blocks — whose Q/K/V deps completed at unit 6 — hiding
                # the chain->activation->perm latency that otherwise stalls
                # the PE at ~20-27us.
                p0g = project(0)
                drain(take(p0g, 6))
                gen00 = attention(0, 0)
                interleave(p0g, take(gen00, 2), 2, 2)
                interleave(
                    project(1), chain2(gen00, attention(1, 0)), 8, 10
                )
                for sc in range(2, NQC - 1):
                    interleave(
                        project(sc), att_both(sc - 1), 8, 2 * (4 * (sc - 1) + 6)
                    )
                gen03 = attention(0, NQC - 1)
                interleave(
                    project(NQC - 1),
                    chain2(att_both(NQC - 2), take(gen03, 12)),
                    8,
                    2 * (4 * (NQC - 2) + 6) + 12,
                )
                drain(gen03)
                exchange(0)
                drain(attention(1, NQC - 1))
                exchange(1)

            # =============================================================
            # Phase C: gathered attn^T -> out projection for my q-slice.
            # p-major: pair 0's accumulation only depends on the first
            # AllToAll, so it overlaps the second one.
            # =============================================================
            with (
                tc.tile_pool(name="out_sb", bufs=4) as osp,
                tc.tile_pool(name="op_psum", bufs=8, space="PSUM") as opp,
            ):
                slots = []  # (psum, b2, sb, nsl)
                for b2 in range(B):
                    for sb in range(SLC // 128):
                        for nc2 in range(2):
                            nsl = slice(512 * nc2, 512 * nc2 + 512)
                            ps = opp.tile(
                                [128, 512], F32, tag="op", name=f"op{b2}{sb}{nc2}"
                            )
                            slots.append((ps, b2, sb, nsl))

                # dependency-free filler matmuls into the last slot's psum
                # (overwritten by its real accumulation below): they run
                # immediately after the last attention matmuls, keeping the
                # PE p-state at full clock through the first-AllToAll wait
                # so phase C p=0 does not start at half speed.
                for _ in range(10):
                    nc.tensor.matmul(
                        slots[-1][0][:],
                        ones_f[:, 0:128],
                        cw_s[:, OFF_COS : OFF_COS + 512],
                        start=True,
                        stop=True,
                        skip_group_check=True,
                    )

                # all bias matmuls first: they depend only on cw_s, so
                # they run during the first-AllToAll wait instead of head-of-
                # line blocking behind each slot's af-dependent matmuls.
                for ps, b2, sb, nsl in slots:
                    nc.tensor.matmul(
                        ps[:],
                        cw_s[0:1, OFF_ROW_ONES : OFF_ROW_ONES + 128],
                        bo_s[:, nsl],
                        start=True,
                        stop=False,
                    )

                for p in range(NP):
                    # (af_all loads for pair p were issued inside exchange(p))
                    for idx, (ps, b2, sb, nsl) in enumerate(slots):
                        for k in range(p, 8, 2):
                            nc.tensor.matmul(
                                ps[:],
                                af_all[:, b2, k, 128 * sb : 128 * sb + 128],
                                wo_v[:, k, nsl],
                                start=False,
                                stop=(p == 1 and k == 7),
                            )
                        if p == 1:
                            # drain+store right behind each slot's last
                            # accumulation so the output pipeline overlaps
                            # the remaining slots' matmuls.
                            o_t = osp.tile([128, 512], BF16, tag="o")
                            if idx % 2 == 0:
                                nc.vector.tensor_copy(o_t[:], ps[:])
                            else:
                                nc.scalar.activation(o_t[:], ps[:], Ident)
                            nc.sync.dma_start(
                                out=out[b2, 128 * sb : 128 * sb + 128, nsl],
                                in_=o_t[:],
                            )
    nc.finalize()
    return nc


_PROGRAM = None


def _get_program():
    global _PROGRAM
    if _PROGRAM is None:
        _PROGRAM = build_program()
    return _PROGRAM


def make_in_maps(x, Wqkv, bqkv, Wout, bout):
    x = np.asarray(x, dtype=np.float32)
    Wqkv = np.asarray(Wqkv, dtype=np.float32)
    bqkv = np.asarray(bqkv, dtype=np.float32)
    Wout = np.asarray(Wout, dtype=np.float32)
    bout = np.asarray(bout, dtype=np.float32)

    cosT, sinT, perm_np, tri_np = _host_constants()
    # x^T per batch, packed [128, sc, c, s]: row p, (sc,c,s) -> x[512sc+s, b, 128c+p]
    xt_b = []
    for b in range(B):
        xtb = x[:, b, :].T                                      # [D, S]
        xtb = xtb.reshape(8, 128, NQC, QC).transpose(1, 2, 0, 3)  # [p, sc, c, s]
        xt_b.append(np.ascontiguousarray(xtb.astype(ml_dtypes.bfloat16)))
    # wout packed [128, k, n]: row p, (k, n) -> Wout[128k+p, n]
    wo_pack = np.ascontiguousarray(
        Wout.reshape(8, 128, D).transpose(1, 0, 2).reshape(128, 8 * D)
    ).astype(ml_dtypes.bfloat16)
    bo = bout.reshape(1, D)

    in_maps = []
    for c in range(8):
        b, g = c // 4, c % 4
        cols = slice(64 * HPC * g, 64 * HPC * (g + 1))  # this core's head dims
        def pack_w(w):  # [D, n] -> [128, 8*n]
            n = w.shape[1]
            return w.reshape(8, 128, n).transpose(1, 0, 2).reshape(128, 8 * n)

        wq_c = Wqkv[:, 0 * D :][:, cols]
        wk_c = Wqkv[:, 1 * D :][:, cols]
        cw_np = np.zeros((128, CW), dtype=np.float32)
        cw_np[:, OFF_WQ0 : OFF_WQ0 + 8 * 128] = pack_w(wq_c[:, 0:128])
        cw_np[:, OFF_WK0 : OFF_WK0 + 8 * 128] = pack_w(wk_c[:, 0:128])
        cw_np[:, OFF_WQ1 : OFF_WQ1 + 8 * 128] = pack_w(wq_c[:, 128:256])
        cw_np[:, OFF_WK1 : OFF_WK1 + 8 * 128] = pack_w(wk_c[:, 128:256])
        cw_np[:, OFF_WV : OFF_WV + 8 * 256] = pack_w(Wqkv[:, 2 * D :][:, cols])
        cw_np[:, OFF_PERM : OFF_PERM + 128] = perm_np
        cw_np[:, OFF_ONES : OFF_ONES + 128] = 1.0
        cw_np[:, OFF_TRIN : OFF_TRIN + 128] = tri_np
        cw_np[:, OFF_COS : OFF_COS + S] = cosT
        cw_np[:, OFF_SIN : OFF_SIN + S] = sinT
        cw_np[0, OFF_ROW_ONES : OFF_ROW_ONES + 512] = 1.0
        cw_np[0, OFF_ROW_BO : OFF_ROW_BO + D] = bo[0]

        bqk_np = np.stack(
            [
                bqkv[0 * D :][cols][0:128],
                bqkv[0 * D :][cols][128:256],
                bqkv[1 * D :][cols][0:128],
                bqkv[1 * D :][cols][128:256],
                np.full((128,), -EXP_SHIFT),
            ],
            axis=1,
        ).astype(np.float32)  # [128, 5]: bq_p0 | bq_p1 | bk_p0 | bk_p1 | -shift

        in_maps.append(
            {
                "xt": xt_b[b],
                "cw": cw_np.astype(ml_dtypes.bfloat16),
                "bqk": bqk_np,
                "wo": wo_pack,
            }
        )
    return in_maps


def unshard(results):
    out = np.empty((S, B, D), dtype=np.float32)
    for r in range(8):
        for b2 in range(B):
            out[SLC * r : SLC * (r + 1), b2, :] = results[r]["out_s"][b2]
    return out


def kernel(x, Wqkv, bqkv, Wout, bout, **_kw):
    nc = _get_program()
    in_maps = make_in_maps(x, Wqkv, bqkv, Wout, bout)
    res = run_bass_kernel_spmd(nc, in_maps, list(range(8)))
    return unshard(res.results)

